# revision 15
# baseline (speedup 1.0000x reference)
"""Trainium2 Bass kernel for nn_KnowledgeBaseModule.

Data-parallel over batch: 8 batch rows -> 8 NeuronCores, weights/tables
replicated. Entity encoder (the dominant compute) runs in feature-major
(transposed) layout so both weight matmuls use natural weight layouts; the
two big matmuls use float32r (fast fp32) operands, everything index-critical
stays fp32.
"""

import numpy as np

import concourse.bass as bass
import concourse.mybir as mybir
import concourse.tile as tile
from concourse import bacc
from concourse.masks import make_identity

f32 = mybir.dt.float32
f32r = mybir.dt.float32r
i32 = mybir.dt.int32
u32 = mybir.dt.uint32

P = 128
H = 1024
H2 = 2048
T = 2048          # tokens per core (one batch row)
TB = 512          # token block
NBLK = T // TB    # 4
NE = 1000
TOPK = 5
EPS = 1e-5
KO1 = H // P      # 8
MO1 = H2 // P     # 16
KO2 = H2 // P     # 16
MO2 = H // P      # 8
N_CORES = 8

AF = mybir.ActivationFunctionType
ALU = mybir.AluOpType


def _row_to_col(nc, ps_pool, one1, col_pool, row, n_chunks, name):
    """[1, n_chunks*128] row -> [128, n_chunks] column layout via K=1 matmuls."""
    ps_col = ps_pool.tile([P, n_chunks], f32, name=f"{name}_ps", tag="colacc")
    for ko in range(n_chunks):
        nc.tensor.matmul(
            ps_col[:, ko : ko + 1],
            lhsT=row[:, ko * P : (ko + 1) * P],
            rhs=one1,
            start=True,
            stop=True,
        )
    col = col_pool.tile([P, n_chunks], f32, name=f"{name}_col", tag=f"{name}_col")
    nc.vector.tensor_copy(col, ps_col)
    return col


def _row_layernorm(nc, rows_pool, scal_pool, row, g_row, b_row, eps1, name):
    """LayerNorm along the free dim of a [1, N] row (single partition)."""
    n = row.shape[1]

    def scal(nm):
        return scal_pool.tile([1, 1], f32, name=f"{name}_{nm}", tag="scal")

    s1 = scal("s1")
    s2 = scal("s2")
    sq = rows_pool.tile([1, n], f32, name=f"{name}_sq", tag="rows")
    nc.vector.tensor_reduce(s1, row, axis=mybir.AxisListType.X, op=ALU.add)
    nc.vector.tensor_tensor(sq, row, row, ALU.mult)
    nc.vector.tensor_reduce(s2, sq, axis=mybir.AxisListType.X, op=ALU.add)
    mean = scal("mean")
    msq = scal("msq")
    nc.scalar.mul(mean, s1, 1.0 / n)
    nc.scalar.mul(msq, s2, 1.0 / n)
    m2 = scal("m2")
    var = scal("var")
    nc.vector.tensor_tensor(m2, mean, mean, ALU.mult)
    nc.vector.tensor_tensor(var, msq, m2, ALU.subtract)
    nc.scalar.activation(var, var, AF.Sqrt, bias=eps1, scale=1.0)
    nc.vector.reciprocal(var, var)
    out = rows_pool.tile([1, n], f32, name=f"{name}_out", tag="rows")
    nc.vector.tensor_scalar(
        out, row, scalar1=mean, scalar2=var, op0=ALU.subtract, op1=ALU.mult
    )
    nc.vector.tensor_tensor(out, out, g_row, ALU.mult)
    nc.vector.tensor_tensor(out, out, b_row, ALU.add)
    return out


def build_kernel():
    import concourse.tile_utils as tile_utils

    tile_utils.max_sbuf_usage = 206 * 1024

    nc = bacc.Bacc()

    # ---- DRAM I/O ----
    x = nc.dram_tensor("x", [T, H], f32, kind="ExternalInput")
    ent = nc.dram_tensor("ent", [NE, H], f32, kind="ExternalInput")
    ee_w1 = nc.dram_tensor("ee_w1", [H, H2], f32, kind="ExternalInput")
    ee_b1 = nc.dram_tensor("ee_b1", [1, H2], f32, kind="ExternalInput")
    ee_w2 = nc.dram_tensor("ee_w2", [H2, H], f32, kind="ExternalInput")
    ee_b2 = nc.dram_tensor("ee_b2", [1, H], f32, kind="ExternalInput")
    ee_g = nc.dram_tensor("ee_g", [1, H], f32, kind="ExternalInput")
    ee_beta = nc.dram_tensor("ee_beta", [1, H], f32, kind="ExternalInput")
    re_w1 = nc.dram_tensor("re_w1", [H, H], f32, kind="ExternalInput")
    re_b1 = nc.dram_tensor("re_b1", [1, H], f32, kind="ExternalInput")
    re_w2 = nc.dram_tensor("re_w2", [H, H], f32, kind="ExternalInput")
    re_b2 = nc.dram_tensor("re_b2", [1, H], f32, kind="ExternalInput")
    re_g = nc.dram_tensor("re_g", [1, H], f32, kind="ExternalInput")
    re_beta = nc.dram_tensor("re_beta", [1, H], f32, kind="ExternalInput")
    rn_w1 = nc.dram_tensor("rn_w1", [3 * H, H2], f32, kind="ExternalInput")
    rn_b1 = nc.dram_tensor("rn_b1", [1, H2], f32, kind="ExternalInput")
    rn_w2 = nc.dram_tensor("rn_w2", [H2, H], f32, kind="ExternalInput")
    rn_b2 = nc.dram_tensor("rn_b2", [1, H], f32, kind="ExternalInput")
    rn_g = nc.dram_tensor("rn_g", [1, H], f32, kind="ExternalInput")
    rn_beta = nc.dram_tensor("rn_beta", [1, H], f32, kind="ExternalInput")
    vn_w1 = nc.dram_tensor("vn_w1", [H, H // 2], f32, kind="ExternalInput")
    vn_b1 = nc.dram_tensor("vn_b1", [1, H // 2], f32, kind="ExternalInput")
    vn_w2 = nc.dram_tensor("vn_w2", [H // 2, 1], f32, kind="ExternalInput")
    vn_b2 = nc.dram_tensor("vn_b2", [1, 1], f32, kind="ExternalInput")

    ef_out = nc.dram_tensor("ef_out", [T, H], f32, kind="ExternalOutput")
    rf_out = nc.dram_tensor("rf_out", [1, H], f32, kind="ExternalOutput")
    ret_out = nc.dram_tensor("ret_out", [TOPK, H], f32, kind="ExternalOutput")
    sims_out = nc.dram_tensor("sims_out", [1, NE], f32, kind="ExternalOutput")
    idx_out = nc.dram_tensor("idx_out", [1, TOPK], i32, kind="ExternalOutput")
    ro_out = nc.dram_tensor("ro_out", [1, H], f32, kind="ExternalOutput")
    vs_out = nc.dram_tensor("vs_out", [1, 1], f32, kind="ExternalOutput")

    from contextlib import ExitStack

    with tile.TileContext(nc) as tc, ExitStack() as stack:
        # ---------- persistent constants ----------
        cpool = stack.enter_context(tc.tile_pool(name="consts", bufs=1))
        ident = cpool.tile([P, P], f32)
        make_identity(nc, ident)
        b1_col = cpool.tile([P, MO1], f32)
        nc.sync.dma_start(out=b1_col, in_=ee_b1.rearrange("a (m p) -> p (a m)", p=P))
        b2_col = cpool.tile([P, MO2], f32)
        nc.sync.dma_start(out=b2_col, in_=ee_b2.rearrange("a (m p) -> p (a m)", p=P))
        g_bc = cpool.tile([P, H], f32)
        nc.sync.dma_start(out=g_bc, in_=ee_g[:, :].to_broadcast([P, H]))
        beta_bc = cpool.tile([P, H], f32)
        nc.sync.dma_start(out=beta_bc, in_=ee_beta[:, :].to_broadcast([P, H]))
        eps_t = cpool.tile([P, 1], f32)
        nc.vector.memset(eps_t, EPS)
        eps1 = cpool.tile([1, 1], f32)
        nc.vector.memset(eps1, EPS)
        ones_row = cpool.tile([1, P], f32)
        nc.vector.memset(ones_row, 1.0)
        pooled_acc = cpool.tile([P, KO1], f32)
        nc.vector.memset(pooled_acc, 0.0)

        # ---------- phase A: entity encoder ----------
        with (
            tc.tile_pool(name="w1p", bufs=1) as w1p,
            tc.tile_pool(name="wstage", bufs=2) as wstage,
            tc.tile_pool(name="w2sp", bufs=2) as w2sp,
            tc.tile_pool(name="w2rp", bufs=3) as w2rp,
            tc.tile_pool(name="xp", bufs=1) as xp,
            tc.tile_pool(name="xtp", bufs=1) as xtp,
            tc.tile_pool(name="htp", bufs=1) as htp,
            tc.tile_pool(name="h2tp", bufs=1) as h2tp,
            tc.tile_pool(name="efp", bufs=2) as efp,
            tc.tile_pool(name="lnp", bufs=4) as lnp,
            tc.tile_pool(name="psmm", bufs=2, space="PSUM") as psmm,
            tc.tile_pool(name="psacc", bufs=4, space="PSUM") as psacc,
            tc.tile_pool(name="pstp", bufs=2, space="PSUM") as pstp,
        ):
            NTT = TB // P  # token subtiles per block

            # W1 resident: DMA f32 stage, round to f32r on DVE
            w1r = []
            for ko in range(KO1):
                stg = wstage.tile([P, H2], f32, name=f"w1s{ko}", tag="wstage")
                nc.sync.dma_start(out=stg, in_=ee_w1[ko * P : (ko + 1) * P, :])
                wr = w1p.tile([P, H2], f32r, name=f"w1r{ko}", tag=f"w1r{ko}")
                nc.vector.tensor_copy(wr, stg)
                w1r.append(wr)

            for b in range(NBLK):
                xa = xp.tile([P, NTT, H], f32, name="xa", tag="xa")
                nc.sync.dma_start(
                    out=xa,
                    in_=x[b * TB : (b + 1) * TB, :].rearrange(
                        "(tt p) h -> p tt h", p=P
                    ),
                )
                # transpose to feature-major (gpsimd evictions round to f32r);
                # pooled accumulated from the exact f32 PSUM tiles on DVE
                xt = xtp.tile([P, KO1, TB], f32r, name="xt", tag="xt")
                red = [
                    lnp.tile([P, KO1], f32, name=f"red{tt}", tag=f"red{tt}")
                    for tt in range(NTT)
                ]
                for tt in range(NTT):
                    for ko in range(KO1):
                        tp_ps = pstp.tile([P, P], f32, name="tp_ps", tag="tp")
                        nc.tensor.transpose(
                            tp_ps, xa[:, tt, ko * P : (ko + 1) * P], ident
                        )
                        nc.vector.tensor_copy(
                            xt[:, ko, tt * P : (tt + 1) * P], tp_ps
                        )
                        nc.vector.tensor_reduce(
                            red[tt][:, ko : ko + 1],
                            tp_ps,
                            axis=mybir.AxisListType.X,
                            op=ALU.add,
                        )
                for tt in range(NTT):
                    nc.vector.tensor_tensor(
                        pooled_acc, pooled_acc, red[tt], ALU.add
                    )

                # mm1 + gelu -> ht (f32r)
                ht = htp.tile([P, MO1, TB], f32r, name="ht", tag="ht")
                for m in range(MO1):
                    ps1 = psmm.tile([P, TB], f32, name="ps1", tag="mm")
                    for ko in range(KO1):
                        nc.tensor.matmul(
                            ps1,
                            lhsT=w1r[ko][:, m * P : (m + 1) * P],
                            rhs=xt[:, ko, :],
                            start=(ko == 0),
                            stop=(ko == KO1 - 1),
                        )
                    nc.scalar.activation(
                        ht[:, m, :], ps1, AF.Gelu, bias=b1_col[:, m : m + 1], scale=1.0
                    )

                # mm2: stream W2 column-halves, 4 psum accumulators
                h2t = h2tp.tile([P, MO2, TB], f32, name="h2t", tag="h2t")
                for g in range(2):
                    accs = [
                        psacc.tile([P, TB], f32, name=f"acc{m2}", tag="acc")
                        for m2 in range(4)
                    ]
                    for ko in range(KO2):
                        w2s = w2sp.tile([P, 512], f32, name="w2s", tag="w2s")
                        nc.sync.dma_start(
                            out=w2s,
                            in_=ee_w2[
                                ko * P : (ko + 1) * P, g * 512 : (g + 1) * 512
                            ],
                        )
                        w2g = w2rp.tile([P, 512], f32r, name="w2g", tag="w2g")
                        nc.vector.tensor_copy(w2g, w2s)
                        for m2 in range(4):
                            nc.tensor.matmul(
                                accs[m2],
                                lhsT=w2g[:, m2 * P : (m2 + 1) * P],
                                rhs=ht[:, ko, :],
                                start=(ko == 0),
                                stop=(ko == KO2 - 1),
                            )
                    for m2 in range(4):
                        nc.vector.tensor_scalar(
                            h2t[:, g * 4 + m2, :],
                            accs[m2],
                            scalar1=b2_col[:, g * 4 + m2 : g * 4 + m2 + 1],
                            scalar2=None,
                            op0=ALU.add,
                        )

                # transpose back to token-major + LayerNorm + store
                for tt in range(NTT):
                    ef_t = efp.tile([P, H], f32, name="ef_t", tag="ef")
                    for mo in range(MO2):
                        tp2 = pstp.tile([P, P], f32, name="tp2", tag="tp")
                        nc.tensor.transpose(
                            tp2, h2t[:, mo, tt * P : (tt + 1) * P], ident
                        )
                        nc.vector.tensor_copy(
                            ef_t[:, mo * P : (mo + 1) * P], tp2
                        )
                    stats = lnp.tile([P, 2, 6], f32, name="stats", tag="stats")
                    for sg in range(2):
                        nc.vector.bn_stats(
                            stats[:, sg, :], ef_t[:, sg * 512 : (sg + 1) * 512]
                        )
                    mv = lnp.tile([P, 2], f32, name="mv", tag="mv")
                    nc.vector.bn_aggr(mv, stats)
                    nc.scalar.activation(
                        mv[:, 1:2], mv[:, 1:2], AF.Sqrt, bias=eps_t, scale=1.0
                    )
                    nc.vector.reciprocal(mv[:, 1:2], mv[:, 1:2])
                    nc.vector.tensor_scalar(
                        ef_t,
                        ef_t,
                        scalar1=mv[:, 0:1],
                        scalar2=mv[:, 1:2],
                        op0=ALU.subtract,
                        op1=ALU.mult,
                    )
                    nc.vector.tensor_tensor(ef_t, ef_t, g_bc, ALU.mult)
                    nc.vector.tensor_tensor(ef_t, ef_t, beta_bc, ALU.add)
                    nc.sync.dma_start(
                        out=ef_out[b * TB + tt * P : b * TB + (tt + 1) * P, :],
                        in_=ef_t,
                    )

        # ---------- phase B: pooled -> relation -> retrieval -> reasoning ----------
        with (
            tc.tile_pool(name="rows", bufs=8) as rows,
            tc.tile_pool(name="scal", bufs=12) as scal_p,
            tc.tile_pool(name="colp", bufs=1) as colp,
            tc.tile_pool(name="entp", bufs=1) as entp,
            tc.tile_pool(name="etp", bufs=1) as etp,
            tc.tile_pool(name="rwp", bufs=1) as rwp,
            tc.tile_pool(name="rnwp", bufs=2) as rnwp,
            tc.tile_pool(name="rnwrp", bufs=2) as rnwrp,
            tc.tile_pool(name="vnwp", bufs=2) as vnwp,
            tc.tile_pool(name="psb_tp", bufs=2, space="PSUM") as psb_tp,
            tc.tile_pool(name="psb_va", bufs=4, space="PSUM") as psb_va,
            tc.tile_pool(name="psb_ca", bufs=2, space="PSUM") as psb_ca,
        ):
            def row_tile(n, nm):
                return rows.tile([1, n], f32, name=nm, tag="rows")

            def load_row(dram, n, nm):
                t_ = row_tile(n, nm)
                nc.sync.dma_start(out=t_, in_=dram[:, :])
                return t_

            one1 = colp.tile([1, 1], f32, name="one1", tag="one1")
            nc.vector.memset(one1, 1.0)

            pooled_col = colp.tile([P, KO1], f32, name="pooled_col", tag="pooled_col")
            nc.scalar.mul(pooled_col, pooled_acc, 1.0 / T)

            # --- relation encoder (row-major, fp32) ---
            rw1 = rwp.tile([P, KO1, H], f32, name="rw1", tag="rew")
            nc.sync.dma_start(out=rw1, in_=re_w1.rearrange("(ko p) n -> p ko n", p=P))
            reb1 = load_row(re_b1, H, "reb1")
            h1_row = row_tile(H, "h1_row")
            for n in range(2):
                ps_a = psb_va.tile([1, 512], f32, name="ps_a", tag="vecacc")
                for ko in range(KO1):
                    nc.tensor.matmul(
                        ps_a,
                        lhsT=pooled_col[:, ko : ko + 1],
                        rhs=rw1[:, ko, n * 512 : (n + 1) * 512],
                        start=(ko == 0),
                        stop=(ko == KO1 - 1),
                    )
                nc.vector.tensor_tensor(
                    h1_row[:, n * 512 : (n + 1) * 512],
                    ps_a,
                    reb1[:, n * 512 : (n + 1) * 512],
                    ALU.add,
                )
            nc.scalar.activation(h1_row, h1_row, AF.Gelu)
            g1_col = _row_to_col(nc, psb_ca, one1, colp, h1_row, KO1, "g1")

            rw2 = rwp.tile([P, KO1, H], f32, name="rw2", tag="rew")
            nc.sync.dma_start(out=rw2, in_=re_w2.rearrange("(ko p) n -> p ko n", p=P))
            reb2 = load_row(re_b2, H, "reb2")
            r_row = row_tile(H, "r_row")
            for n in range(2):
                ps_a = psb_va.tile([1, 512], f32, name="ps_a2", tag="vecacc")
                for ko in range(KO1):
                    nc.tensor.matmul(
                        ps_a,
                        lhsT=g1_col[:, ko : ko + 1],
                        rhs=rw2[:, ko, n * 512 : (n + 1) * 512],
                        start=(ko == 0),
                        stop=(ko == KO1 - 1),
                    )
                nc.vector.tensor_tensor(
                    r_row[:, n * 512 : (n + 1) * 512],
                    ps_a,
                    reb2[:, n * 512 : (n + 1) * 512],
                    ALU.add,
                )
            reg_row = load_row(re_g, H, "reg_row")
            rebeta_row = load_row(re_beta, H, "rebeta_row")
            rf_row = _row_layernorm(
                nc, rows, scal_p, r_row, reg_row, rebeta_row, eps1, "rf"
            )
            nc.sync.dma_start(out=rf_out[:, :], in_=rf_row)
            rf_col = _row_to_col(nc, psb_ca, one1, colp, rf_row, KO1, "rf")

            # --- entity table: load + transpose ---
            ent_nat = []
            for et in range(8):
                rows_e = P if et < 7 else NE - 7 * P
                t_ = entp.tile([P, H], f32, name=f"ent{et}", tag=f"ent{et}")
                nc.sync.dma_start(
                    out=t_[:rows_e, :], in_=ent[et * P : et * P + rows_e, :]
                )
                ent_nat.append((t_, rows_e))
            et_t = etp.tile([P, KO1, NE], f32)
            for et in range(8):
                t_, rows_e = ent_nat[et]
                for fo in range(KO1):
                    tpE = psb_tp.tile([P, P], f32, name="tpE", tag="tp")
                    nc.tensor.transpose(
                        tpE[:, :rows_e],
                        t_[:rows_e, fo * P : (fo + 1) * P],
                        ident[:rows_e, :rows_e],
                    )
                    nc.vector.tensor_copy(
                        et_t[:, fo, et * P : et * P + rows_e], tpE[:, :rows_e]
                    )

            # --- similarities + top-k ---
            sims_row = row_tile(NE, "sims_row")
            for n in range(2):
                nsz = 512 if n == 0 else NE - 512
                ps_s = psb_va.tile([1, 512], f32, name="ps_s", tag="vecacc")
                for ko in range(KO1):
                    nc.tensor.matmul(
                        ps_s[:, :nsz],
                        lhsT=rf_col[:, ko : ko + 1],
                        rhs=et_t[:, ko, n * 512 : n * 512 + nsz],
                        start=(ko == 0),
                        stop=(ko == KO1 - 1),
                    )
                nc.vector.tensor_copy(
                    sims_row[:, n * 512 : n * 512 + nsz], ps_s[:, :nsz]
                )
            nc.sync.dma_start(out=sims_out[:, :], in_=sims_row)

            mxv = colp.tile([1, 8], f32, name="mxv", tag="mxv")
            mxi = colp.tile([1, 8], u32, name="mxi", tag="mxi")
            nc.vector.max_with_indices(mxv, mxi, sims_row)
            mii = colp.tile([1, 8], i32, name="mii", tag="mii")
            nc.vector.tensor_copy(mii, mxi)
            nc.sync.dma_start(out=idx_out[:, :], in_=mii[:, :TOPK])

            # --- gather retrieved entities via one-hot matmul ---
            mif = colp.tile([1, 8], f32, name="mif", tag="mif")
            nc.vector.tensor_copy(mif, mxi)
            ps_ib = psb_ca.tile([P, 8], f32, name="ps_ib", tag="colacc")
            nc.tensor.matmul(ps_ib, lhsT=ones_row, rhs=mif, start=True, stop=True)
            idx_b = colp.tile([P, 8], f32, name="idx_b", tag="idx_b")
            nc.vector.tensor_copy(idx_b, ps_ib)
            iota_i = colp.tile([P, 8], i32, name="iota_i", tag="iota_i")
            nc.gpsimd.iota(iota_i, pattern=[[P, 8]], base=0, channel_multiplier=1)
            iota_f = colp.tile([P, 8], f32, name="iota_f", tag="iota_f")
            nc.vector.tensor_copy(iota_f, iota_i)
            onehot = colp.tile([P, 8, TOPK], f32, name="onehot", tag="onehot")
            for et in range(8):
                nc.vector.tensor_tensor(
                    onehot[:, et, :],
                    idx_b[:, :TOPK],
                    iota_f[:, et : et + 1].to_broadcast([P, TOPK]),
                    ALU.is_equal,
                )
            ret_col = colp.tile([P, KO1, TOPK], f32, name="ret_col", tag="ret_col")
            for fo in range(KO1):
                ps_g = psb_ca.tile([P, TOPK], f32, name="ps_g", tag="colacc")
                for et in range(8):
                    t_, rows_e = ent_nat[et]
                    nc.tensor.matmul(
                        ps_g,
                        lhsT=t_[:rows_e, fo * P : (fo + 1) * P],
                        rhs=onehot[:rows_e, et, :],
                        start=(et == 0),
                        stop=(et == 7),
                    )
                nc.vector.tensor_copy(ret_col[:, fo, :], ps_g)
            ret_row = rows.tile([TOPK, H], f32, name="ret_row", tag="rows")
            for fo in range(KO1):
                ps_r5 = psb_tp.tile([TOPK, P], f32, name="ps_r5", tag="tp")
                nc.tensor.transpose(ps_r5, ret_col[:, fo, :], ident)
                nc.vector.tensor_copy(ret_row[:, fo * P : (fo + 1) * P], ps_r5)
            nc.sync.dma_start(out=ret_out[:, :], in_=ret_row)
            ev_col = colp.tile([P, KO1], f32, name="ev_col", tag="ev_col")
            nc.vector.tensor_reduce(
                ev_col, ret_col, axis=mybir.AxisListType.X, op=ALU.add
            )
            nc.scalar.mul(ev_col, ev_col, 1.0 / TOPK)

            # --- reasoning network (f32r) ---
            rin_col = colp.tile([P, 24], f32r, name="rin_col", tag="rin_col")
            nc.vector.tensor_copy(rin_col[:, 0:8], ev_col)
            nc.vector.tensor_copy(rin_col[:, 8:16], rf_col)
            nc.vector.tensor_copy(rin_col[:, 16:24], pooled_col)

            def rin_chunk(k):
                return rin_col[:, k : k + 1]

            rnb1 = load_row(rn_b1, H2, "rnb1")
            h1r_row = row_tile(H2, "h1r_row")
            ps_rn = [
                psb_va.tile([1, 512], f32, name=f"ps_rn{n}", tag="vecacc")
                for n in range(4)
            ]
            for ko in range(24):
                rw = rnwp.tile([P, H2], f32, name="rnw", tag="rnw")
                nc.sync.dma_start(out=rw, in_=rn_w1[ko * P : (ko + 1) * P, :])
                rwr = rnwrp.tile([P, H2], f32r, name="rnwr", tag="rnwr")
                nc.vector.tensor_copy(rwr, rw)
                for n in range(4):
                    nc.tensor.matmul(
                        ps_rn[n],
                        lhsT=rin_chunk(ko),
                        rhs=rwr[:, n * 512 : (n + 1) * 512],
                        start=(ko == 0),
                        stop=(ko == 23),
                    )
            for n in range(4):
                nc.vector.tensor_tensor(
                    h1r_row[:, n * 512 : (n + 1) * 512],
                    ps_rn[n],
                    rnb1[:, n * 512 : (n + 1) * 512],
                    ALU.add,
                )
            nc.scalar.activation(h1r_row, h1r_row, AF.Gelu)
            g1r_col = _row_to_col(nc, psb_ca, one1, colp, h1r_row, 16, "g1r")
            g1r_r = colp.tile([P, 16], f32r, name="g1r_r", tag="g1r_r")
            nc.vector.tensor_copy(g1r_r, g1r_col)

            rnb2 = load_row(rn_b2, H, "rnb2")
            r2_row = row_tile(H, "r2_row")
            ps_rn2 = [
                psb_va.tile([1, 512], f32, name=f"ps_rn2{n}", tag="vecacc")
                for n in range(2)
            ]
            for ko in range(16):
                rw = rnwp.tile([P, H], f32, name="rnw2", tag="rnw")
                nc.sync.dma_start(out=rw, in_=rn_w2[ko * P : (ko + 1) * P, :])
                rwr = rnwrp.tile([P, H], f32r, name="rnwr2", tag="rnwr")
                nc.vector.tensor_copy(rwr, rw)
                for n in range(2):
                    nc.tensor.matmul(
                        ps_rn2[n],
                        lhsT=g1r_r[:, ko : ko + 1],
                        rhs=rwr[:, n * 512 : (n + 1) * 512],
                        start=(ko == 0),
                        stop=(ko == 15),
                    )
            for n in range(2):
                nc.vector.tensor_tensor(
                    r2_row[:, n * 512 : (n + 1) * 512],
                    ps_rn2[n],
                    rnb2[:, n * 512 : (n + 1) * 512],
                    ALU.add,
                )
            rng_row = load_row(rn_g, H, "rng_row")
            rnbeta_row = load_row(rn_beta, H, "rnbeta_row")
            ro_row = _row_layernorm(
                nc, rows, scal_p, r2_row, rng_row, rnbeta_row, eps1, "ro"
            )
            nc.sync.dma_start(out=ro_out[:, :], in_=ro_row)

            # --- validation network ---
            ro_col = _row_to_col(nc, psb_ca, one1, colp, ro_row, KO1, "roc")
            vnb1_row = load_row(vn_b1, H // 2, "vnb1_row")
            ps_v = psb_va.tile([1, 512], f32, name="ps_v", tag="vecacc")
            for ko in range(KO1):
                vw = vnwp.tile([P, 512], f32, name="vnw", tag="vnw")
                nc.sync.dma_start(out=vw, in_=vn_w1[ko * P : (ko + 1) * P, :])
                nc.tensor.matmul(
                    ps_v,
                    lhsT=ro_col[:, ko : ko + 1],
                    rhs=vw,
                    start=(ko == 0),
                    stop=(ko == KO1 - 1),
                )
            v1_row = row_tile(512, "v1_row")
            nc.vector.tensor_tensor(v1_row, ps_v, vnb1_row, ALU.add)
            nc.scalar.activation(v1_row, v1_row, AF.Gelu)
            v1_col = _row_to_col(nc, psb_ca, one1, colp, v1_row, 4, "v1")
            vnw2 = colp.tile([P, 4, 1], f32, name="vnw2", tag="vnw2")
            nc.sync.dma_start(
                out=vnw2, in_=vn_w2.rearrange("(ko p) n -> p ko n", p=P)
            )
            vnb2_row = scal_p.tile([1, 1], f32, name="vnb2_row", tag="scal")
            nc.sync.dma_start(out=vnb2_row, in_=vn_b2[:, :])
            ps_v2 = psb_va.tile([1, 1], f32, name="ps_v2", tag="vecacc")
            for ko in range(4):
                nc.tensor.matmul(
                    ps_v2,
                    lhsT=v1_col[:, ko : ko + 1],
                    rhs=vnw2[:, ko, :],
                    start=(ko == 0),
                    stop=(ko == 3),
                )
            v2_row = scal_p.tile([1, 1], f32, name="v2_row", tag="scal")
            nc.vector.tensor_tensor(v2_row, ps_v2, vnb2_row, ALU.add)
            nc.scalar.activation(v2_row, v2_row, AF.Sigmoid)
            nc.sync.dma_start(out=vs_out[:, :], in_=v2_row)

    nc.compile()
    return nc


_NC_CACHE = None


def _get_nc():
    global _NC_CACHE
    if _NC_CACHE is None:
        _NC_CACHE = build_kernel()
    return _NC_CACHE


def run(inputs, trace=False):
    from concourse.bass_utils import run_bass_kernel_spmd

    nc = _get_nc()
    hs = np.ascontiguousarray(inputs["hidden_states"], dtype=np.float32)
    B = hs.shape[0]
    assert B == N_CORES

    def row(name, n):
        return np.ascontiguousarray(
            np.asarray(inputs[name], dtype=np.float32).reshape(1, n)
        )

    shared = {
        "ent": np.ascontiguousarray(inputs["entity_embeddings"], np.float32),
        "ee_w1": np.ascontiguousarray(inputs["ee_w1"], np.float32),
        "ee_b1": row("ee_b1", H2),
        "ee_w2": np.ascontiguousarray(inputs["ee_w2"], np.float32),
        "ee_b2": row("ee_b2", H),
        "ee_g": row("ee_g", H),
        "ee_beta": row("ee_beta", H),
        "re_w1": np.ascontiguousarray(inputs["re_w1"], np.float32),
        "re_b1": row("re_b1", H),
        "re_w2": np.ascontiguousarray(inputs["re_w2"], np.float32),
        "re_b2": row("re_b2", H),
        "re_g": row("re_g", H),
        "re_beta": row("re_beta", H),
        "rn_w1": np.ascontiguousarray(inputs["rn_w1"], np.float32),
        "rn_b1": row("rn_b1", H2),
        "rn_w2": np.ascontiguousarray(inputs["rn_w2"], np.float32),
        "rn_b2": row("rn_b2", H),
        "rn_g": row("rn_g", H),
        "rn_beta": row("rn_beta", H),
        "vn_w1": np.ascontiguousarray(inputs["vn_w1"], np.float32),
        "vn_b1": row("vn_b1", H // 2),
        "vn_w2": np.ascontiguousarray(inputs["vn_w2"], np.float32),
        "vn_b2": row("vn_b2", 1),
    }
    in_maps = [dict(shared, x=np.ascontiguousarray(hs[c])) for c in range(B)]
    res = run_bass_kernel_spmd(
        nc, in_maps, core_ids=list(range(N_CORES)), trace=trace
    )
    r = res.results
    ef = np.stack([r[c]["ef_out"] for c in range(B)])
    rf = np.stack([r[c]["rf_out"][0] for c in range(B)])
    ret = np.stack([r[c]["ret_out"] for c in range(B)])
    sims = np.stack([r[c]["sims_out"][0] for c in range(B)])
    idx = np.stack([r[c]["idx_out"][0] for c in range(B)]).astype(np.int32)
    ro = np.stack([r[c]["ro_out"][0] for c in range(B)])
    vs = np.stack([r[c]["vs_out"][0] for c in range(B)])
    out = (ef, rf, ret, sims, idx, ro, vs)
    return (out, res) if trace else out


def kernel(**inputs):
    return run(inputs, trace=False)


# revision 16
# speedup vs baseline: 1.1945x; 1.1945x over previous
"""Trainium2 Bass kernel for nn_KnowledgeBaseModule.

Data-parallel over batch: 8 batch rows -> 8 NeuronCores, weights/tables
replicated. Entity encoder (the dominant compute) runs in feature-major
(transposed) layout so both weight matmuls use natural weight layouts; the
two big matmuls use float32r (fast fp32) operands, everything index-critical
stays fp32.
"""

import numpy as np

import concourse.bass as bass
import concourse.mybir as mybir
import concourse.tile as tile
from concourse import bacc
from concourse.masks import make_identity

f32 = mybir.dt.float32
f32r = mybir.dt.float32r
i32 = mybir.dt.int32
u32 = mybir.dt.uint32

P = 128
H = 1024
H2 = 2048
T = 2048          # tokens per core (one batch row)
TB = 512          # token block
NBLK = T // TB    # 4
NE = 1000
TOPK = 5
EPS = 1e-5
KO1 = H // P      # 8
MO1 = H2 // P     # 16
KO2 = H2 // P     # 16
MO2 = H // P      # 8
N_CORES = 8

AF = mybir.ActivationFunctionType
ALU = mybir.AluOpType


def _row_to_col(nc, ps_pool, one1, col_pool, row, n_chunks, name):
    """[1, n_chunks*128] row -> [128, n_chunks] column layout via K=1 matmuls."""
    ps_col = ps_pool.tile([P, n_chunks], f32, name=f"{name}_ps", tag="colacc")
    for ko in range(n_chunks):
        nc.tensor.matmul(
            ps_col[:, ko : ko + 1],
            lhsT=row[:, ko * P : (ko + 1) * P],
            rhs=one1,
            start=True,
            stop=True,
        )
    col = col_pool.tile([P, n_chunks], f32, name=f"{name}_col", tag=f"{name}_col")
    nc.vector.tensor_copy(col, ps_col)
    return col


def _row_layernorm(nc, rows_pool, scal_pool, row, g_row, b_row, eps1, name):
    """LayerNorm along the free dim of a [1, N] row (single partition)."""
    n = row.shape[1]

    def scal(nm):
        return scal_pool.tile([1, 1], f32, name=f"{name}_{nm}", tag="scal")

    s1 = scal("s1")
    s2 = scal("s2")
    sq = rows_pool.tile([1, n], f32, name=f"{name}_sq", tag="rows")
    nc.vector.tensor_reduce(s1, row, axis=mybir.AxisListType.X, op=ALU.add)
    nc.vector.tensor_tensor(sq, row, row, ALU.mult)
    nc.vector.tensor_reduce(s2, sq, axis=mybir.AxisListType.X, op=ALU.add)
    mean = scal("mean")
    msq = scal("msq")
    nc.scalar.mul(mean, s1, 1.0 / n)
    nc.scalar.mul(msq, s2, 1.0 / n)
    m2 = scal("m2")
    var = scal("var")
    nc.vector.tensor_tensor(m2, mean, mean, ALU.mult)
    nc.vector.tensor_tensor(var, msq, m2, ALU.subtract)
    nc.scalar.activation(var, var, AF.Sqrt, bias=eps1, scale=1.0)
    nc.vector.reciprocal(var, var)
    out = rows_pool.tile([1, n], f32, name=f"{name}_out", tag="rows")
    nc.vector.tensor_scalar(
        out, row, scalar1=mean, scalar2=var, op0=ALU.subtract, op1=ALU.mult
    )
    nc.vector.tensor_tensor(out, out, g_row, ALU.mult)
    nc.vector.tensor_tensor(out, out, b_row, ALU.add)
    return out


def build_kernel():
    import concourse.tile_utils as tile_utils

    tile_utils.max_sbuf_usage = 206 * 1024

    nc = bacc.Bacc()

    # ---- DRAM I/O ----
    x = nc.dram_tensor("x", [T, H], f32, kind="ExternalInput")
    ent = nc.dram_tensor("ent", [NE, H], f32, kind="ExternalInput")
    ee_w1 = nc.dram_tensor("ee_w1", [H, H2], f32, kind="ExternalInput")
    ee_b1 = nc.dram_tensor("ee_b1", [1, H2], f32, kind="ExternalInput")
    ee_w2 = nc.dram_tensor("ee_w2", [H2, H], f32, kind="ExternalInput")
    ee_b2 = nc.dram_tensor("ee_b2", [1, H], f32, kind="ExternalInput")
    ee_g = nc.dram_tensor("ee_g", [1, H], f32, kind="ExternalInput")
    ee_beta = nc.dram_tensor("ee_beta", [1, H], f32, kind="ExternalInput")
    re_w1 = nc.dram_tensor("re_w1", [H, H], f32, kind="ExternalInput")
    re_b1 = nc.dram_tensor("re_b1", [1, H], f32, kind="ExternalInput")
    re_w2 = nc.dram_tensor("re_w2", [H, H], f32, kind="ExternalInput")
    re_b2 = nc.dram_tensor("re_b2", [1, H], f32, kind="ExternalInput")
    re_g = nc.dram_tensor("re_g", [1, H], f32, kind="ExternalInput")
    re_beta = nc.dram_tensor("re_beta", [1, H], f32, kind="ExternalInput")
    rn_w1 = nc.dram_tensor("rn_w1", [3 * H, H2], f32, kind="ExternalInput")
    rn_b1 = nc.dram_tensor("rn_b1", [1, H2], f32, kind="ExternalInput")
    rn_w2 = nc.dram_tensor("rn_w2", [H2, H], f32, kind="ExternalInput")
    rn_b2 = nc.dram_tensor("rn_b2", [1, H], f32, kind="ExternalInput")
    rn_g = nc.dram_tensor("rn_g", [1, H], f32, kind="ExternalInput")
    rn_beta = nc.dram_tensor("rn_beta", [1, H], f32, kind="ExternalInput")
    vn_w1 = nc.dram_tensor("vn_w1", [H, H // 2], f32, kind="ExternalInput")
    vn_b1 = nc.dram_tensor("vn_b1", [1, H // 2], f32, kind="ExternalInput")
    vn_w2 = nc.dram_tensor("vn_w2", [H // 2, 1], f32, kind="ExternalInput")
    vn_b2 = nc.dram_tensor("vn_b2", [1, 1], f32, kind="ExternalInput")

    ef_out = nc.dram_tensor("ef_out", [T, H], f32, kind="ExternalOutput")
    rf_out = nc.dram_tensor("rf_out", [1, H], f32, kind="ExternalOutput")
    ret_out = nc.dram_tensor("ret_out", [TOPK, H], f32, kind="ExternalOutput")
    sims_out = nc.dram_tensor("sims_out", [1, NE], f32, kind="ExternalOutput")
    idx_out = nc.dram_tensor("idx_out", [1, TOPK], i32, kind="ExternalOutput")
    ro_out = nc.dram_tensor("ro_out", [1, H], f32, kind="ExternalOutput")
    vs_out = nc.dram_tensor("vs_out", [1, 1], f32, kind="ExternalOutput")

    from contextlib import ExitStack

    with tile.TileContext(nc) as tc, ExitStack() as stack:
        # ---------- persistent constants ----------
        cpool = stack.enter_context(tc.tile_pool(name="consts", bufs=1))
        ident = cpool.tile([P, P], f32)
        make_identity(nc, ident)
        b1_col = cpool.tile([P, MO1], f32)
        nc.sync.dma_start(out=b1_col, in_=ee_b1.rearrange("a (m p) -> p (a m)", p=P))
        b2_col = cpool.tile([P, MO2], f32)
        nc.sync.dma_start(out=b2_col, in_=ee_b2.rearrange("a (m p) -> p (a m)", p=P))
        g_bc = cpool.tile([P, H], f32)
        nc.sync.dma_start(out=g_bc, in_=ee_g[:, :].to_broadcast([P, H]))
        beta_bc = cpool.tile([P, H], f32)
        nc.sync.dma_start(out=beta_bc, in_=ee_beta[:, :].to_broadcast([P, H]))
        eps_t = cpool.tile([P, 1], f32)
        nc.vector.memset(eps_t, EPS)
        eps1 = cpool.tile([1, 1], f32)
        nc.vector.memset(eps1, EPS)
        ones_row = cpool.tile([1, P], f32)
        nc.vector.memset(ones_row, 1.0)
        pooled_acc = cpool.tile([P, KO1], f32)
        nc.vector.memset(pooled_acc, 0.0)

        # ---------- phase A: entity encoder ----------
        with (
            tc.tile_pool(name="w1p", bufs=1) as w1p,
            tc.tile_pool(name="w2rp", bufs=3) as w2rp,
            tc.tile_pool(name="xp", bufs=1) as xp,
            tc.tile_pool(name="xtp", bufs=1) as xtp,
            tc.tile_pool(name="htp", bufs=1) as htp,
            tc.tile_pool(name="h2tp", bufs=1) as h2tp,
            tc.tile_pool(name="efp", bufs=2) as efp,
            tc.tile_pool(name="lnp", bufs=4) as lnp,
            tc.tile_pool(name="psmm", bufs=2, space="PSUM") as psmm,
            tc.tile_pool(name="psacc", bufs=4, space="PSUM") as psacc,
            tc.tile_pool(name="pstp", bufs=2, space="PSUM") as pstp,
        ):
            NTT = TB // P  # token subtiles per block

            # W1 resident as f32r: DMA direct (PE rounds f32r operands on load)
            w1r = []
            for ko in range(KO1):
                wr = w1p.tile([P, H2], f32r, name=f"w1r{ko}", tag=f"w1r{ko}")
                nc.sync.dma_start(
                    out=wr, in_=ee_w1[ko * P : (ko + 1) * P, :].bitcast(f32r)
                )
                w1r.append(wr)

            for b in range(NBLK):
                xa = xp.tile([P, NTT, H], f32, name="xa", tag="xa")
                nc.sync.dma_start(
                    out=xa,
                    in_=x[b * TB : (b + 1) * TB, :].rearrange(
                        "(tt p) h -> p tt h", p=P
                    ),
                )
                # transpose to feature-major (gpsimd evictions round to f32r);
                # pooled accumulated from the exact f32 PSUM tiles on DVE
                xt = xtp.tile([P, KO1, TB], f32r, name="xt", tag="xt")
                red = [
                    lnp.tile([P, KO1], f32, name=f"red{tt}", tag=f"red{tt}")
                    for tt in range(NTT)
                ]
                for tt in range(NTT):
                    for ko in range(KO1):
                        tp_ps = pstp.tile([P, P], f32, name="tp_ps", tag="tp")
                        nc.tensor.transpose(
                            tp_ps, xa[:, tt, ko * P : (ko + 1) * P], ident
                        )
                        nc.vector.tensor_copy(
                            xt[:, ko, tt * P : (tt + 1) * P], tp_ps
                        )
                        nc.vector.tensor_reduce(
                            red[tt][:, ko : ko + 1],
                            tp_ps,
                            axis=mybir.AxisListType.X,
                            op=ALU.add,
                        )
                for tt in range(NTT):
                    nc.vector.tensor_tensor(
                        pooled_acc, pooled_acc, red[tt], ALU.add
                    )

                # mm1 + gelu -> ht (f32r)
                ht = htp.tile([P, MO1, TB], f32r, name="ht", tag="ht")
                for m in range(MO1):
                    ps1 = psmm.tile([P, TB], f32, name="ps1", tag="mm")
                    for ko in range(KO1):
                        nc.tensor.matmul(
                            ps1,
                            lhsT=w1r[ko][:, m * P : (m + 1) * P],
                            rhs=xt[:, ko, :],
                            start=(ko == 0),
                            stop=(ko == KO1 - 1),
                        )
                    nc.scalar.activation(
                        ht[:, m, :], ps1, AF.Gelu, bias=b1_col[:, m : m + 1], scale=1.0
                    )

                # mm2: stream W2 column-halves, 4 psum accumulators
                h2t = h2tp.tile([P, MO2, TB], f32, name="h2t", tag="h2t")
                for g in range(2):
                    accs = [
                        psacc.tile([P, TB], f32, name=f"acc{m2}", tag="acc")
                        for m2 in range(4)
                    ]
                    for ko in range(KO2):
                        w2g = w2rp.tile([P, 512], f32r, name="w2g", tag="w2g")
                        nc.sync.dma_start(
                            out=w2g,
                            in_=ee_w2[
                                ko * P : (ko + 1) * P, g * 512 : (g + 1) * 512
                            ].bitcast(f32r),
                        )
                        for m2 in range(4):
                            nc.tensor.matmul(
                                accs[m2],
                                lhsT=w2g[:, m2 * P : (m2 + 1) * P],
                                rhs=ht[:, ko, :],
                                start=(ko == 0),
                                stop=(ko == KO2 - 1),
                            )
                    for m2 in range(4):
                        nc.vector.tensor_scalar(
                            h2t[:, g * 4 + m2, :],
                            accs[m2],
                            scalar1=b2_col[:, g * 4 + m2 : g * 4 + m2 + 1],
                            scalar2=None,
                            op0=ALU.add,
                        )

                # transpose back to token-major + LayerNorm + store
                for tt in range(NTT):
                    ef_t = efp.tile([P, H], f32, name="ef_t", tag="ef")
                    for mo in range(MO2):
                        tp2 = pstp.tile([P, P], f32, name="tp2", tag="tp")
                        nc.tensor.transpose(
                            tp2, h2t[:, mo, tt * P : (tt + 1) * P], ident
                        )
                        nc.vector.tensor_copy(
                            ef_t[:, mo * P : (mo + 1) * P], tp2
                        )
                    stats = lnp.tile([P, 2, 6], f32, name="stats", tag="stats")
                    for sg in range(2):
                        nc.vector.bn_stats(
                            stats[:, sg, :], ef_t[:, sg * 512 : (sg + 1) * 512]
                        )
                    mv = lnp.tile([P, 2], f32, name="mv", tag="mv")
                    nc.vector.bn_aggr(mv, stats)
                    nc.scalar.activation(
                        mv[:, 1:2], mv[:, 1:2], AF.Sqrt, bias=eps_t, scale=1.0
                    )
                    nc.vector.reciprocal(mv[:, 1:2], mv[:, 1:2])
                    nc.vector.tensor_scalar(
                        ef_t,
                        ef_t,
                        scalar1=mv[:, 0:1],
                        scalar2=mv[:, 1:2],
                        op0=ALU.subtract,
                        op1=ALU.mult,
                    )
                    nc.vector.tensor_tensor(ef_t, ef_t, g_bc, ALU.mult)
                    nc.vector.tensor_tensor(ef_t, ef_t, beta_bc, ALU.add)
                    nc.sync.dma_start(
                        out=ef_out[b * TB + tt * P : b * TB + (tt + 1) * P, :],
                        in_=ef_t,
                    )

        # ---------- phase B: pooled -> relation -> retrieval -> reasoning ----------
        with (
            tc.tile_pool(name="rows", bufs=8) as rows,
            tc.tile_pool(name="scal", bufs=12) as scal_p,
            tc.tile_pool(name="colp", bufs=1) as colp,
            tc.tile_pool(name="entp", bufs=1) as entp,
            tc.tile_pool(name="etp", bufs=1) as etp,
            tc.tile_pool(name="rwp", bufs=1) as rwp,
            tc.tile_pool(name="rnwp", bufs=3) as rnwp,
            tc.tile_pool(name="vnwp", bufs=2) as vnwp,
            tc.tile_pool(name="psb_tp", bufs=2, space="PSUM") as psb_tp,
            tc.tile_pool(name="psb_va", bufs=4, space="PSUM") as psb_va,
            tc.tile_pool(name="psb_ca", bufs=2, space="PSUM") as psb_ca,
        ):
            def row_tile(n, nm):
                return rows.tile([1, n], f32, name=nm, tag="rows")

            def load_row(dram, n, nm):
                t_ = row_tile(n, nm)
                nc.sync.dma_start(out=t_, in_=dram[:, :])
                return t_

            one1 = colp.tile([1, 1], f32, name="one1", tag="one1")
            nc.vector.memset(one1, 1.0)

            pooled_col = colp.tile([P, KO1], f32, name="pooled_col", tag="pooled_col")
            nc.scalar.mul(pooled_col, pooled_acc, 1.0 / T)

            # --- relation encoder (row-major, fp32) ---
            rw1 = rwp.tile([P, KO1, H], f32, name="rw1", tag="rew")
            nc.sync.dma_start(out=rw1, in_=re_w1.rearrange("(ko p) n -> p ko n", p=P))
            reb1 = load_row(re_b1, H, "reb1")
            h1_row = row_tile(H, "h1_row")
            for n in range(2):
                ps_a = psb_va.tile([1, 512], f32, name="ps_a", tag="vecacc")
                for ko in range(KO1):
                    nc.tensor.matmul(
                        ps_a,
                        lhsT=pooled_col[:, ko : ko + 1],
                        rhs=rw1[:, ko, n * 512 : (n + 1) * 512],
                        start=(ko == 0),
                        stop=(ko == KO1 - 1),
                    )
                nc.vector.tensor_tensor(
                    h1_row[:, n * 512 : (n + 1) * 512],
                    ps_a,
                    reb1[:, n * 512 : (n + 1) * 512],
                    ALU.add,
                )
            nc.scalar.activation(h1_row, h1_row, AF.Gelu)
            g1_col = _row_to_col(nc, psb_ca, one1, colp, h1_row, KO1, "g1")

            rw2 = rwp.tile([P, KO1, H], f32, name="rw2", tag="rew")
            nc.sync.dma_start(out=rw2, in_=re_w2.rearrange("(ko p) n -> p ko n", p=P))
            reb2 = load_row(re_b2, H, "reb2")
            r_row = row_tile(H, "r_row")
            for n in range(2):
                ps_a = psb_va.tile([1, 512], f32, name="ps_a2", tag="vecacc")
                for ko in range(KO1):
                    nc.tensor.matmul(
                        ps_a,
                        lhsT=g1_col[:, ko : ko + 1],
                        rhs=rw2[:, ko, n * 512 : (n + 1) * 512],
                        start=(ko == 0),
                        stop=(ko == KO1 - 1),
                    )
                nc.vector.tensor_tensor(
                    r_row[:, n * 512 : (n + 1) * 512],
                    ps_a,
                    reb2[:, n * 512 : (n + 1) * 512],
                    ALU.add,
                )
            reg_row = load_row(re_g, H, "reg_row")
            rebeta_row = load_row(re_beta, H, "rebeta_row")
            rf_row = _row_layernorm(
                nc, rows, scal_p, r_row, reg_row, rebeta_row, eps1, "rf"
            )
            nc.sync.dma_start(out=rf_out[:, :], in_=rf_row)
            rf_col = _row_to_col(nc, psb_ca, one1, colp, rf_row, KO1, "rf")

            # --- entity table: load + transpose ---
            ent_nat = []
            for et in range(8):
                rows_e = P if et < 7 else NE - 7 * P
                t_ = entp.tile([P, H], f32, name=f"ent{et}", tag=f"ent{et}")
                nc.sync.dma_start(
                    out=t_[:rows_e, :], in_=ent[et * P : et * P + rows_e, :]
                )
                ent_nat.append((t_, rows_e))
            et_t = etp.tile([P, KO1, NE], f32)
            for et in range(8):
                t_, rows_e = ent_nat[et]
                for fo in range(KO1):
                    tpE = psb_tp.tile([P, P], f32, name="tpE", tag="tp")
                    nc.tensor.transpose(
                        tpE[:, :rows_e],
                        t_[:rows_e, fo * P : (fo + 1) * P],
                        ident[:rows_e, :rows_e],
                    )
                    nc.vector.tensor_copy(
                        et_t[:, fo, et * P : et * P + rows_e], tpE[:, :rows_e]
                    )

            # --- similarities + top-k ---
            sims_row = row_tile(NE, "sims_row")
            for n in range(2):
                nsz = 512 if n == 0 else NE - 512
                ps_s = psb_va.tile([1, 512], f32, name="ps_s", tag="vecacc")
                for ko in range(KO1):
                    nc.tensor.matmul(
                        ps_s[:, :nsz],
                        lhsT=rf_col[:, ko : ko + 1],
                        rhs=et_t[:, ko, n * 512 : n * 512 + nsz],
                        start=(ko == 0),
                        stop=(ko == KO1 - 1),
                    )
                nc.vector.tensor_copy(
                    sims_row[:, n * 512 : n * 512 + nsz], ps_s[:, :nsz]
                )
            nc.sync.dma_start(out=sims_out[:, :], in_=sims_row)

            mxv = colp.tile([1, 8], f32, name="mxv", tag="mxv")
            mxi = colp.tile([1, 8], u32, name="mxi", tag="mxi")
            nc.vector.max_with_indices(mxv, mxi, sims_row)
            mii = colp.tile([1, 8], i32, name="mii", tag="mii")
            nc.vector.tensor_copy(mii, mxi)
            nc.sync.dma_start(out=idx_out[:, :], in_=mii[:, :TOPK])

            # --- gather retrieved entities via one-hot matmul ---
            mif = colp.tile([1, 8], f32, name="mif", tag="mif")
            nc.vector.tensor_copy(mif, mxi)
            ps_ib = psb_ca.tile([P, 8], f32, name="ps_ib", tag="colacc")
            nc.tensor.matmul(ps_ib, lhsT=ones_row, rhs=mif, start=True, stop=True)
            idx_b = colp.tile([P, 8], f32, name="idx_b", tag="idx_b")
            nc.vector.tensor_copy(idx_b, ps_ib)
            iota_i = colp.tile([P, 8], i32, name="iota_i", tag="iota_i")
            nc.gpsimd.iota(iota_i, pattern=[[P, 8]], base=0, channel_multiplier=1)
            iota_f = colp.tile([P, 8], f32, name="iota_f", tag="iota_f")
            nc.vector.tensor_copy(iota_f, iota_i)
            onehot = colp.tile([P, 8, TOPK], f32, name="onehot", tag="onehot")
            for et in range(8):
                nc.vector.tensor_tensor(
                    onehot[:, et, :],
                    idx_b[:, :TOPK],
                    iota_f[:, et : et + 1].to_broadcast([P, TOPK]),
                    ALU.is_equal,
                )
            ret_col = colp.tile([P, KO1, TOPK], f32, name="ret_col", tag="ret_col")
            for fo in range(KO1):
                ps_g = psb_ca.tile([P, TOPK], f32, name="ps_g", tag="colacc")
                for et in range(8):
                    t_, rows_e = ent_nat[et]
                    nc.tensor.matmul(
                        ps_g,
                        lhsT=t_[:rows_e, fo * P : (fo + 1) * P],
                        rhs=onehot[:rows_e, et, :],
                        start=(et == 0),
                        stop=(et == 7),
                    )
                nc.vector.tensor_copy(ret_col[:, fo, :], ps_g)
            ret_row = rows.tile([TOPK, H], f32, name="ret_row", tag="rows")
            for fo in range(KO1):
                ps_r5 = psb_tp.tile([TOPK, P], f32, name="ps_r5", tag="tp")
                nc.tensor.transpose(ps_r5, ret_col[:, fo, :], ident)
                nc.vector.tensor_copy(ret_row[:, fo * P : (fo + 1) * P], ps_r5)
            nc.sync.dma_start(out=ret_out[:, :], in_=ret_row)
            ev_col = colp.tile([P, KO1], f32, name="ev_col", tag="ev_col")
            nc.vector.tensor_reduce(
                ev_col, ret_col, axis=mybir.AxisListType.X, op=ALU.add
            )
            nc.scalar.mul(ev_col, ev_col, 1.0 / TOPK)

            # --- reasoning network (f32r) ---
            rin_col = colp.tile([P, 24], f32r, name="rin_col", tag="rin_col")
            nc.vector.tensor_copy(rin_col[:, 0:8], ev_col)
            nc.vector.tensor_copy(rin_col[:, 8:16], rf_col)
            nc.vector.tensor_copy(rin_col[:, 16:24], pooled_col)

            def rin_chunk(k):
                return rin_col[:, k : k + 1]

            rnb1 = load_row(rn_b1, H2, "rnb1")
            h1r_row = row_tile(H2, "h1r_row")
            ps_rn = [
                psb_va.tile([1, 512], f32, name=f"ps_rn{n}", tag="vecacc")
                for n in range(4)
            ]
            for ko in range(24):
                rwr = rnwp.tile([P, H2], f32r, name="rnw", tag="rnw")
                nc.sync.dma_start(
                    out=rwr, in_=rn_w1[ko * P : (ko + 1) * P, :].bitcast(f32r)
                )
                for n in range(4):
                    nc.tensor.matmul(
                        ps_rn[n],
                        lhsT=rin_chunk(ko),
                        rhs=rwr[:, n * 512 : (n + 1) * 512],
                        start=(ko == 0),
                        stop=(ko == 23),
                    )
            for n in range(4):
                nc.vector.tensor_tensor(
                    h1r_row[:, n * 512 : (n + 1) * 512],
                    ps_rn[n],
                    rnb1[:, n * 512 : (n + 1) * 512],
                    ALU.add,
                )
            nc.scalar.activation(h1r_row, h1r_row, AF.Gelu)
            g1r_col = _row_to_col(nc, psb_ca, one1, colp, h1r_row, 16, "g1r")
            g1r_r = colp.tile([P, 16], f32r, name="g1r_r", tag="g1r_r")
            nc.vector.tensor_copy(g1r_r, g1r_col)

            rnb2 = load_row(rn_b2, H, "rnb2")
            r2_row = row_tile(H, "r2_row")
            ps_rn2 = [
                psb_va.tile([1, 512], f32, name=f"ps_rn2{n}", tag="vecacc")
                for n in range(2)
            ]
            for ko in range(16):
                rwr = rnwp.tile([P, H], f32r, name="rnw2", tag="rnw")
                nc.sync.dma_start(
                    out=rwr, in_=rn_w2[ko * P : (ko + 1) * P, :].bitcast(f32r)
                )
                for n in range(2):
                    nc.tensor.matmul(
                        ps_rn2[n],
                        lhsT=g1r_r[:, ko : ko + 1],
                        rhs=rwr[:, n * 512 : (n + 1) * 512],
                        start=(ko == 0),
                        stop=(ko == 15),
                    )
            for n in range(2):
                nc.vector.tensor_tensor(
                    r2_row[:, n * 512 : (n + 1) * 512],
                    ps_rn2[n],
                    rnb2[:, n * 512 : (n + 1) * 512],
                    ALU.add,
                )
            rng_row = load_row(rn_g, H, "rng_row")
            rnbeta_row = load_row(rn_beta, H, "rnbeta_row")
            ro_row = _row_layernorm(
                nc, rows, scal_p, r2_row, rng_row, rnbeta_row, eps1, "ro"
            )
            nc.sync.dma_start(out=ro_out[:, :], in_=ro_row)

            # --- validation network ---
            ro_col = _row_to_col(nc, psb_ca, one1, colp, ro_row, KO1, "roc")
            vnb1_row = load_row(vn_b1, H // 2, "vnb1_row")
            ps_v = psb_va.tile([1, 512], f32, name="ps_v", tag="vecacc")
            for ko in range(KO1):
                vw = vnwp.tile([P, 512], f32, name="vnw", tag="vnw")
                nc.sync.dma_start(out=vw, in_=vn_w1[ko * P : (ko + 1) * P, :])
                nc.tensor.matmul(
                    ps_v,
                    lhsT=ro_col[:, ko : ko + 1],
                    rhs=vw,
                    start=(ko == 0),
                    stop=(ko == KO1 - 1),
                )
            v1_row = row_tile(512, "v1_row")
            nc.vector.tensor_tensor(v1_row, ps_v, vnb1_row, ALU.add)
            nc.scalar.activation(v1_row, v1_row, AF.Gelu)
            v1_col = _row_to_col(nc, psb_ca, one1, colp, v1_row, 4, "v1")
            vnw2 = colp.tile([P, 4, 1], f32, name="vnw2", tag="vnw2")
            nc.sync.dma_start(
                out=vnw2, in_=vn_w2.rearrange("(ko p) n -> p ko n", p=P)
            )
            vnb2_row = scal_p.tile([1, 1], f32, name="vnb2_row", tag="scal")
            nc.sync.dma_start(out=vnb2_row, in_=vn_b2[:, :])
            ps_v2 = psb_va.tile([1, 1], f32, name="ps_v2", tag="vecacc")
            for ko in range(4):
                nc.tensor.matmul(
                    ps_v2,
                    lhsT=v1_col[:, ko : ko + 1],
                    rhs=vnw2[:, ko, :],
                    start=(ko == 0),
                    stop=(ko == 3),
                )
            v2_row = scal_p.tile([1, 1], f32, name="v2_row", tag="scal")
            nc.vector.tensor_tensor(v2_row, ps_v2, vnb2_row, ALU.add)
            nc.scalar.activation(v2_row, v2_row, AF.Sigmoid)
            nc.sync.dma_start(out=vs_out[:, :], in_=v2_row)

    nc.compile()
    return nc


_NC_CACHE = None


def _get_nc():
    global _NC_CACHE
    if _NC_CACHE is None:
        _NC_CACHE = build_kernel()
    return _NC_CACHE


def run(inputs, trace=False):
    from concourse.bass_utils import run_bass_kernel_spmd

    nc = _get_nc()
    hs = np.ascontiguousarray(inputs["hidden_states"], dtype=np.float32)
    B = hs.shape[0]
    assert B == N_CORES

    def row(name, n):
        return np.ascontiguousarray(
            np.asarray(inputs[name], dtype=np.float32).reshape(1, n)
        )

    shared = {
        "ent": np.ascontiguousarray(inputs["entity_embeddings"], np.float32),
        "ee_w1": np.ascontiguousarray(inputs["ee_w1"], np.float32),
        "ee_b1": row("ee_b1", H2),
        "ee_w2": np.ascontiguousarray(inputs["ee_w2"], np.float32),
        "ee_b2": row("ee_b2", H),
        "ee_g": row("ee_g", H),
        "ee_beta": row("ee_beta", H),
        "re_w1": np.ascontiguousarray(inputs["re_w1"], np.float32),
        "re_b1": row("re_b1", H),
        "re_w2": np.ascontiguousarray(inputs["re_w2"], np.float32),
        "re_b2": row("re_b2", H),
        "re_g": row("re_g", H),
        "re_beta": row("re_beta", H),
        "rn_w1": np.ascontiguousarray(inputs["rn_w1"], np.float32),
        "rn_b1": row("rn_b1", H2),
        "rn_w2": np.ascontiguousarray(inputs["rn_w2"], np.float32),
        "rn_b2": row("rn_b2", H),
        "rn_g": row("rn_g", H),
        "rn_beta": row("rn_beta", H),
        "vn_w1": np.ascontiguousarray(inputs["vn_w1"], np.float32),
        "vn_b1": row("vn_b1", H // 2),
        "vn_w2": np.ascontiguousarray(inputs["vn_w2"], np.float32),
        "vn_b2": row("vn_b2", 1),
    }
    in_maps = [dict(shared, x=np.ascontiguousarray(hs[c])) for c in range(B)]
    res = run_bass_kernel_spmd(
        nc, in_maps, core_ids=list(range(N_CORES)), trace=trace
    )
    r = res.results
    ef = np.stack([r[c]["ef_out"] for c in range(B)])
    rf = np.stack([r[c]["rf_out"][0] for c in range(B)])
    ret = np.stack([r[c]["ret_out"] for c in range(B)])
    sims = np.stack([r[c]["sims_out"][0] for c in range(B)])
    idx = np.stack([r[c]["idx_out"][0] for c in range(B)]).astype(np.int32)
    ro = np.stack([r[c]["ro_out"][0] for c in range(B)])
    vs = np.stack([r[c]["vs_out"][0] for c in range(B)])
    out = (ef, rf, ret, sims, idx, ro, vs)
    return (out, res) if trace else out


def kernel(**inputs):
    return run(inputs, trace=False)


# revision 17
# speedup vs baseline: 1.4929x; 1.2498x over previous
"""Trainium2 Bass kernel for nn_KnowledgeBaseModule.

Data-parallel over batch: 8 batch rows -> 8 NeuronCores, weights/tables
replicated. Entity encoder (the dominant compute) runs in feature-major
(transposed) layout so both weight matmuls use natural weight layouts; the
two big matmuls use float32r (fast fp32) operands, everything index-critical
stays fp32.
"""

import numpy as np

import concourse.bass as bass
import concourse.mybir as mybir
import concourse.tile as tile
from concourse import bacc
from concourse.masks import make_identity

f32 = mybir.dt.float32
f32r = mybir.dt.float32r
i32 = mybir.dt.int32
u32 = mybir.dt.uint32

P = 128
H = 1024
H2 = 2048
T = 2048          # tokens per core (one batch row)
TB = 512          # token block
NBLK = T // TB    # 4
NE = 1000
TOPK = 5
EPS = 1e-5
KO1 = H // P      # 8
MO1 = H2 // P     # 16
KO2 = H2 // P     # 16
MO2 = H // P      # 8
N_CORES = 8

AF = mybir.ActivationFunctionType
ALU = mybir.AluOpType


def _row_to_col(nc, ps_pool, one1, col_pool, row, n_chunks, name):
    """[1, n_chunks*128] row -> [128, n_chunks] column layout via K=1 matmuls."""
    ps_col = ps_pool.tile([P, n_chunks], f32, name=f"{name}_ps", tag="colacc")
    for ko in range(n_chunks):
        nc.tensor.matmul(
            ps_col[:, ko : ko + 1],
            lhsT=row[:, ko * P : (ko + 1) * P],
            rhs=one1,
            start=True,
            stop=True,
        )
    col = col_pool.tile([P, n_chunks], f32, name=f"{name}_col", tag=f"{name}_col")
    nc.vector.tensor_copy(col, ps_col)
    return col


def _row_layernorm(nc, rows_pool, scal_pool, row, g_row, b_row, eps1, name):
    """LayerNorm along the free dim of a [1, N] row (single partition)."""
    n = row.shape[1]

    def scal(nm):
        return scal_pool.tile([1, 1], f32, name=f"{name}_{nm}", tag="scal")

    s1 = scal("s1")
    s2 = scal("s2")
    sq = rows_pool.tile([1, n], f32, name=f"{name}_sq", tag="rows")
    nc.vector.tensor_reduce(s1, row, axis=mybir.AxisListType.X, op=ALU.add)
    nc.vector.tensor_tensor(sq, row, row, ALU.mult)
    nc.vector.tensor_reduce(s2, sq, axis=mybir.AxisListType.X, op=ALU.add)
    mean = scal("mean")
    msq = scal("msq")
    nc.scalar.mul(mean, s1, 1.0 / n)
    nc.scalar.mul(msq, s2, 1.0 / n)
    m2 = scal("m2")
    var = scal("var")
    nc.vector.tensor_tensor(m2, mean, mean, ALU.mult)
    nc.vector.tensor_tensor(var, msq, m2, ALU.subtract)
    nc.scalar.activation(var, var, AF.Sqrt, bias=eps1, scale=1.0)
    nc.vector.reciprocal(var, var)
    out = rows_pool.tile([1, n], f32, name=f"{name}_out", tag="rows")
    nc.vector.tensor_scalar(
        out, row, scalar1=mean, scalar2=var, op0=ALU.subtract, op1=ALU.mult
    )
    nc.vector.tensor_tensor(out, out, g_row, ALU.mult)
    nc.vector.tensor_tensor(out, out, b_row, ALU.add)
    return out


def build_kernel():
    import concourse.tile_utils as tile_utils

    tile_utils.max_sbuf_usage = 206 * 1024

    nc = bacc.Bacc()

    # ---- DRAM I/O ----
    xT = nc.dram_tensor("xT", [H, T], f32, kind="ExternalInput")
    ent = nc.dram_tensor("ent", [NE, H], f32, kind="ExternalInput")
    entT = nc.dram_tensor("entT", [H, NE], f32, kind="ExternalInput")
    ee_w1 = nc.dram_tensor("ee_w1", [H, H2], f32, kind="ExternalInput")
    ee_b1 = nc.dram_tensor("ee_b1", [1, H2], f32, kind="ExternalInput")
    ee_w2 = nc.dram_tensor("ee_w2", [H2, H], f32, kind="ExternalInput")
    ee_b2 = nc.dram_tensor("ee_b2", [1, H], f32, kind="ExternalInput")
    ee_g = nc.dram_tensor("ee_g", [1, H], f32, kind="ExternalInput")
    ee_beta = nc.dram_tensor("ee_beta", [1, H], f32, kind="ExternalInput")
    re_w1 = nc.dram_tensor("re_w1", [H, H], f32, kind="ExternalInput")
    re_b1 = nc.dram_tensor("re_b1", [1, H], f32, kind="ExternalInput")
    re_w2 = nc.dram_tensor("re_w2", [H, H], f32, kind="ExternalInput")
    re_b2 = nc.dram_tensor("re_b2", [1, H], f32, kind="ExternalInput")
    re_g = nc.dram_tensor("re_g", [1, H], f32, kind="ExternalInput")
    re_beta = nc.dram_tensor("re_beta", [1, H], f32, kind="ExternalInput")
    rn_w1 = nc.dram_tensor("rn_w1", [3 * H, H2], f32, kind="ExternalInput")
    rn_b1 = nc.dram_tensor("rn_b1", [1, H2], f32, kind="ExternalInput")
    rn_w2 = nc.dram_tensor("rn_w2", [H2, H], f32, kind="ExternalInput")
    rn_b2 = nc.dram_tensor("rn_b2", [1, H], f32, kind="ExternalInput")
    rn_g = nc.dram_tensor("rn_g", [1, H], f32, kind="ExternalInput")
    rn_beta = nc.dram_tensor("rn_beta", [1, H], f32, kind="ExternalInput")
    vn_w1 = nc.dram_tensor("vn_w1", [H, H // 2], f32, kind="ExternalInput")
    vn_b1 = nc.dram_tensor("vn_b1", [1, H // 2], f32, kind="ExternalInput")
    vn_w2 = nc.dram_tensor("vn_w2", [H // 2, 1], f32, kind="ExternalInput")
    vn_b2 = nc.dram_tensor("vn_b2", [1, 1], f32, kind="ExternalInput")

    ef_out = nc.dram_tensor("ef_out", [T, H], f32, kind="ExternalOutput")
    rf_out = nc.dram_tensor("rf_out", [1, H], f32, kind="ExternalOutput")
    ret_out = nc.dram_tensor("ret_out", [TOPK, H], f32, kind="ExternalOutput")
    sims_out = nc.dram_tensor("sims_out", [1, NE], f32, kind="ExternalOutput")
    idx_out = nc.dram_tensor("idx_out", [1, TOPK], i32, kind="ExternalOutput")
    ro_out = nc.dram_tensor("ro_out", [1, H], f32, kind="ExternalOutput")
    vs_out = nc.dram_tensor("vs_out", [1, 1], f32, kind="ExternalOutput")

    from contextlib import ExitStack

    with tile.TileContext(nc) as tc, ExitStack() as stack:
        # ---------- persistent constants ----------
        cpool = stack.enter_context(tc.tile_pool(name="consts", bufs=1))
        ident = cpool.tile([P, P], f32)
        make_identity(nc, ident)
        b1_col = cpool.tile([P, MO1], f32)
        nc.sync.dma_start(out=b1_col, in_=ee_b1.rearrange("a (m p) -> p (a m)", p=P))
        b2_bc = cpool.tile([P, H], f32)
        nc.sync.dma_start(out=b2_bc, in_=ee_b2[:, :].to_broadcast([P, H]))
        et_t = cpool.tile([P, KO1, NE], f32)
        nc.sync.dma_start(
            out=et_t, in_=entT.rearrange("(ko p) e -> p ko e", p=P)
        )
        g_bc = cpool.tile([P, H], f32)
        nc.sync.dma_start(out=g_bc, in_=ee_g[:, :].to_broadcast([P, H]))
        beta_bc = cpool.tile([P, H], f32)
        nc.sync.dma_start(out=beta_bc, in_=ee_beta[:, :].to_broadcast([P, H]))
        eps_t = cpool.tile([P, 1], f32)
        nc.vector.memset(eps_t, EPS)
        eps1 = cpool.tile([1, 1], f32)
        nc.vector.memset(eps1, EPS)
        ones_row = cpool.tile([1, P], f32)
        nc.vector.memset(ones_row, 1.0)
        pooled_acc = cpool.tile([P, KO1], f32)
        nc.vector.memset(pooled_acc, 0.0)

        # ---------- phase A: entity encoder ----------
        with (
            tc.tile_pool(name="w1p", bufs=1) as w1p,
            tc.tile_pool(name="w2rp", bufs=3) as w2rp,
            tc.tile_pool(name="xtp", bufs=2) as xtp,
            tc.tile_pool(name="htp", bufs=1) as htp,
            tc.tile_pool(name="efp", bufs=5) as efp,
            tc.tile_pool(name="lnp", bufs=4) as lnp,
            tc.tile_pool(name="psmm", bufs=3, space="PSUM") as psmm,
            tc.tile_pool(name="psacc", bufs=4, space="PSUM") as psacc,
        ):
            NTT = TB // P  # token subtiles per block

            # W1 resident as f32r: DMA direct (PE rounds f32r operands on load)
            w1r = []
            for ko in range(KO1):
                wr = w1p.tile([P, H2], f32r, name=f"w1r{ko}", tag=f"w1r{ko}")
                nc.sync.dma_start(
                    out=wr, in_=ee_w1[ko * P : (ko + 1) * P, :].bitcast(f32r)
                )
                w1r.append(wr)

            for b in range(NBLK):
                # X.T block direct from DRAM (host-pretransposed); f32r tag is
                # only seen by the PE — DVE reads the same bits as exact f32.
                xt = xtp.tile([P, KO1, TB], f32r, name="xt", tag="xt")
                nc.sync.dma_start(
                    out=xt,
                    in_=xT[:, b * TB : (b + 1) * TB]
                    .rearrange("(ko p) t -> p ko t", p=P)
                    .bitcast(f32r),
                )
                red = lnp.tile([P, KO1], f32, name="red", tag="red")
                nc.vector.tensor_reduce(
                    red,
                    xt.bitcast(f32),
                    axis=mybir.AxisListType.X,
                    op=ALU.add,
                )
                nc.vector.tensor_tensor(pooled_acc, pooled_acc, red, ALU.add)

                # mm1 + gelu -> ht (f32r, feature-major)
                ht = htp.tile([P, MO1, TB], f32r, name="ht", tag="ht")
                for m in range(MO1):
                    ps1 = psmm.tile([P, TB], f32, name="ps1", tag="mm")
                    for ko in range(KO1):
                        nc.tensor.matmul(
                            ps1,
                            lhsT=w1r[ko][:, m * P : (m + 1) * P],
                            rhs=xt[:, ko, :],
                            start=(ko == 0),
                            stop=(ko == KO1 - 1),
                        )
                    nc.scalar.activation(
                        ht[:, m, :], ps1, AF.Gelu, bias=b1_col[:, m : m + 1], scale=1.0
                    )

                # mm2 token-major: lhsT = ht chunks, rhs = streamed W2 rows
                ef_blk = [
                    efp.tile([P, H], f32, name=f"ef{tt}", tag="ef")
                    for tt in range(NTT)
                ]
                for g in range(2):
                    accs = [
                        psacc.tile([P, 512], f32, name=f"acc{tt}", tag="acc")
                        for tt in range(NTT)
                    ]
                    for ko in range(KO2):
                        w2g = w2rp.tile([P, 512], f32r, name="w2g", tag="w2g")
                        nc.sync.dma_start(
                            out=w2g,
                            in_=ee_w2[
                                ko * P : (ko + 1) * P, g * 512 : (g + 1) * 512
                            ].bitcast(f32r),
                        )
                        for tt in range(NTT):
                            nc.tensor.matmul(
                                accs[tt],
                                lhsT=ht[:, ko, tt * P : (tt + 1) * P],
                                rhs=w2g,
                                start=(ko == 0),
                                stop=(ko == KO2 - 1),
                            )
                    for tt in range(NTT):
                        nc.vector.tensor_tensor(
                            ef_blk[tt][:, g * 512 : (g + 1) * 512],
                            accs[tt],
                            b2_bc[:, g * 512 : (g + 1) * 512],
                            ALU.add,
                        )

                # LayerNorm + store (token-major already)
                for tt in range(NTT):
                    ef_t = ef_blk[tt]
                    stats = lnp.tile([P, 2, 6], f32, name="stats", tag="stats")
                    for sg in range(2):
                        nc.vector.bn_stats(
                            stats[:, sg, :], ef_t[:, sg * 512 : (sg + 1) * 512]
                        )
                    mv = lnp.tile([P, 2], f32, name="mv", tag="mv")
                    nc.vector.bn_aggr(mv, stats)
                    nc.scalar.activation(
                        mv[:, 1:2], mv[:, 1:2], AF.Sqrt, bias=eps_t, scale=1.0
                    )
                    nc.vector.reciprocal(mv[:, 1:2], mv[:, 1:2])
                    nc.vector.tensor_scalar(
                        ef_t,
                        ef_t,
                        scalar1=mv[:, 0:1],
                        scalar2=mv[:, 1:2],
                        op0=ALU.subtract,
                        op1=ALU.mult,
                    )
                    nc.vector.tensor_tensor(ef_t, ef_t, g_bc, ALU.mult)
                    nc.vector.tensor_tensor(ef_t, ef_t, beta_bc, ALU.add)
                    nc.sync.dma_start(
                        out=ef_out[b * TB + tt * P : b * TB + (tt + 1) * P, :],
                        in_=ef_t,
                    )

        # ---------- phase B: pooled -> relation -> retrieval -> reasoning ----------
        with (
            tc.tile_pool(name="rows", bufs=8) as rows,
            tc.tile_pool(name="scal", bufs=12) as scal_p,
            tc.tile_pool(name="colp", bufs=1) as colp,
            tc.tile_pool(name="entp", bufs=1) as entp,
            tc.tile_pool(name="rwp", bufs=1) as rwp,
            tc.tile_pool(name="rnwp", bufs=3) as rnwp,
            tc.tile_pool(name="vnwp", bufs=2) as vnwp,
            tc.tile_pool(name="psb_tp", bufs=2, space="PSUM") as psb_tp,
            tc.tile_pool(name="psb_va", bufs=4, space="PSUM") as psb_va,
            tc.tile_pool(name="psb_ca", bufs=2, space="PSUM") as psb_ca,
        ):
            def row_tile(n, nm):
                return rows.tile([1, n], f32, name=nm, tag="rows")

            def load_row(dram, n, nm):
                t_ = row_tile(n, nm)
                nc.sync.dma_start(out=t_, in_=dram[:, :])
                return t_

            one1 = colp.tile([1, 1], f32, name="one1", tag="one1")
            nc.vector.memset(one1, 1.0)

            pooled_col = colp.tile([P, KO1], f32, name="pooled_col", tag="pooled_col")
            nc.scalar.mul(pooled_col, pooled_acc, 1.0 / T)

            # --- relation encoder (row-major, fp32) ---
            rw1 = rwp.tile([P, KO1, H], f32, name="rw1", tag="rew")
            nc.sync.dma_start(out=rw1, in_=re_w1.rearrange("(ko p) n -> p ko n", p=P))
            reb1 = load_row(re_b1, H, "reb1")
            h1_row = row_tile(H, "h1_row")
            for n in range(2):
                ps_a = psb_va.tile([1, 512], f32, name="ps_a", tag="vecacc")
                for ko in range(KO1):
                    nc.tensor.matmul(
                        ps_a,
                        lhsT=pooled_col[:, ko : ko + 1],
                        rhs=rw1[:, ko, n * 512 : (n + 1) * 512],
                        start=(ko == 0),
                        stop=(ko == KO1 - 1),
                    )
                nc.vector.tensor_tensor(
                    h1_row[:, n * 512 : (n + 1) * 512],
                    ps_a,
                    reb1[:, n * 512 : (n + 1) * 512],
                    ALU.add,
                )
            nc.scalar.activation(h1_row, h1_row, AF.Gelu)
            g1_col = _row_to_col(nc, psb_ca, one1, colp, h1_row, KO1, "g1")

            rw2 = rwp.tile([P, KO1, H], f32, name="rw2", tag="rew")
            nc.sync.dma_start(out=rw2, in_=re_w2.rearrange("(ko p) n -> p ko n", p=P))
            reb2 = load_row(re_b2, H, "reb2")
            r_row = row_tile(H, "r_row")
            for n in range(2):
                ps_a = psb_va.tile([1, 512], f32, name="ps_a2", tag="vecacc")
                for ko in range(KO1):
                    nc.tensor.matmul(
                        ps_a,
                        lhsT=g1_col[:, ko : ko + 1],
                        rhs=rw2[:, ko, n * 512 : (n + 1) * 512],
                        start=(ko == 0),
                        stop=(ko == KO1 - 1),
                    )
                nc.vector.tensor_tensor(
                    r_row[:, n * 512 : (n + 1) * 512],
                    ps_a,
                    reb2[:, n * 512 : (n + 1) * 512],
                    ALU.add,
                )
            reg_row = load_row(re_g, H, "reg_row")
            rebeta_row = load_row(re_beta, H, "rebeta_row")
            rf_row = _row_layernorm(
                nc, rows, scal_p, r_row, reg_row, rebeta_row, eps1, "rf"
            )
            nc.sync.dma_start(out=rf_out[:, :], in_=rf_row)
            rf_col = _row_to_col(nc, psb_ca, one1, colp, rf_row, KO1, "rf")

            # --- entity table natural layout (for the gather) ---
            ent_nat = []
            for et in range(8):
                rows_e = P if et < 7 else NE - 7 * P
                t_ = entp.tile([P, H], f32, name=f"ent{et}", tag=f"ent{et}")
                nc.sync.dma_start(
                    out=t_[:rows_e, :], in_=ent[et * P : et * P + rows_e, :]
                )
                ent_nat.append((t_, rows_e))

            # --- similarities + top-k ---
            sims_row = row_tile(NE, "sims_row")
            for n in range(2):
                nsz = 512 if n == 0 else NE - 512
                ps_s = psb_va.tile([1, 512], f32, name="ps_s", tag="vecacc")
                for ko in range(KO1):
                    nc.tensor.matmul(
                        ps_s[:, :nsz],
                        lhsT=rf_col[:, ko : ko + 1],
                        rhs=et_t[:, ko, n * 512 : n * 512 + nsz],
                        start=(ko == 0),
                        stop=(ko == KO1 - 1),
                    )
                nc.vector.tensor_copy(
                    sims_row[:, n * 512 : n * 512 + nsz], ps_s[:, :nsz]
                )
            nc.sync.dma_start(out=sims_out[:, :], in_=sims_row)

            mxv = colp.tile([1, 8], f32, name="mxv", tag="mxv")
            mxi = colp.tile([1, 8], u32, name="mxi", tag="mxi")
            nc.vector.max_with_indices(mxv, mxi, sims_row)
            mii = colp.tile([1, 8], i32, name="mii", tag="mii")
            nc.vector.tensor_copy(mii, mxi)
            nc.sync.dma_start(out=idx_out[:, :], in_=mii[:, :TOPK])

            # --- gather retrieved entities via one-hot matmul ---
            mif = colp.tile([1, 8], f32, name="mif", tag="mif")
            nc.vector.tensor_copy(mif, mxi)
            ps_ib = psb_ca.tile([P, 8], f32, name="ps_ib", tag="colacc")
            nc.tensor.matmul(ps_ib, lhsT=ones_row, rhs=mif, start=True, stop=True)
            idx_b = colp.tile([P, 8], f32, name="idx_b", tag="idx_b")
            nc.vector.tensor_copy(idx_b, ps_ib)
            iota_i = colp.tile([P, 8], i32, name="iota_i", tag="iota_i")
            nc.gpsimd.iota(iota_i, pattern=[[P, 8]], base=0, channel_multiplier=1)
            iota_f = colp.tile([P, 8], f32, name="iota_f", tag="iota_f")
            nc.vector.tensor_copy(iota_f, iota_i)
            onehot = colp.tile([P, 8, TOPK], f32, name="onehot", tag="onehot")
            for et in range(8):
                nc.vector.tensor_tensor(
                    onehot[:, et, :],
                    idx_b[:, :TOPK],
                    iota_f[:, et : et + 1].to_broadcast([P, TOPK]),
                    ALU.is_equal,
                )
            ret_col = colp.tile([P, KO1, TOPK], f32, name="ret_col", tag="ret_col")
            for fo in range(KO1):
                ps_g = psb_ca.tile([P, TOPK], f32, name="ps_g", tag="colacc")
                for et in range(8):
                    t_, rows_e = ent_nat[et]
                    nc.tensor.matmul(
                        ps_g,
                        lhsT=t_[:rows_e, fo * P : (fo + 1) * P],
                        rhs=onehot[:rows_e, et, :],
                        start=(et == 0),
                        stop=(et == 7),
                    )
                nc.vector.tensor_copy(ret_col[:, fo, :], ps_g)
            ret_row = rows.tile([TOPK, H], f32, name="ret_row", tag="rows")
            for fo in range(KO1):
                ps_r5 = psb_tp.tile([TOPK, P], f32, name="ps_r5", tag="tp")
                nc.tensor.transpose(ps_r5, ret_col[:, fo, :], ident)
                nc.vector.tensor_copy(ret_row[:, fo * P : (fo + 1) * P], ps_r5)
            nc.sync.dma_start(out=ret_out[:, :], in_=ret_row)
            ev_col = colp.tile([P, KO1], f32, name="ev_col", tag="ev_col")
            nc.vector.tensor_reduce(
                ev_col, ret_col, axis=mybir.AxisListType.X, op=ALU.add
            )
            nc.scalar.mul(ev_col, ev_col, 1.0 / TOPK)

            # --- reasoning network (f32r) ---
            rin_col = colp.tile([P, 24], f32r, name="rin_col", tag="rin_col")
            nc.vector.tensor_copy(rin_col[:, 0:8], ev_col)
            nc.vector.tensor_copy(rin_col[:, 8:16], rf_col)
            nc.vector.tensor_copy(rin_col[:, 16:24], pooled_col)

            def rin_chunk(k):
                return rin_col[:, k : k + 1]

            rnb1 = load_row(rn_b1, H2, "rnb1")
            h1r_row = row_tile(H2, "h1r_row")
            ps_rn = [
                psb_va.tile([1, 512], f32, name=f"ps_rn{n}", tag="vecacc")
                for n in range(4)
            ]
            for ko in range(24):
                rwr = rnwp.tile([P, H2], f32r, name="rnw", tag="rnw")
                nc.sync.dma_start(
                    out=rwr, in_=rn_w1[ko * P : (ko + 1) * P, :].bitcast(f32r)
                )
                for n in range(4):
                    nc.tensor.matmul(
                        ps_rn[n],
                        lhsT=rin_chunk(ko),
                        rhs=rwr[:, n * 512 : (n + 1) * 512],
                        start=(ko == 0),
                        stop=(ko == 23),
                    )
            for n in range(4):
                nc.vector.tensor_tensor(
                    h1r_row[:, n * 512 : (n + 1) * 512],
                    ps_rn[n],
                    rnb1[:, n * 512 : (n + 1) * 512],
                    ALU.add,
                )
            nc.scalar.activation(h1r_row, h1r_row, AF.Gelu)
            g1r_col = _row_to_col(nc, psb_ca, one1, colp, h1r_row, 16, "g1r")
            g1r_r = colp.tile([P, 16], f32r, name="g1r_r", tag="g1r_r")
            nc.vector.tensor_copy(g1r_r, g1r_col)

            rnb2 = load_row(rn_b2, H, "rnb2")
            r2_row = row_tile(H, "r2_row")
            ps_rn2 = [
                psb_va.tile([1, 512], f32, name=f"ps_rn2{n}", tag="vecacc")
                for n in range(2)
            ]
            for ko in range(16):
                rwr = rnwp.tile([P, H], f32r, name="rnw2", tag="rnw")
                nc.sync.dma_start(
                    out=rwr, in_=rn_w2[ko * P : (ko + 1) * P, :].bitcast(f32r)
                )
                for n in range(2):
                    nc.tensor.matmul(
                        ps_rn2[n],
                        lhsT=g1r_r[:, ko : ko + 1],
                        rhs=rwr[:, n * 512 : (n + 1) * 512],
                        start=(ko == 0),
                        stop=(ko == 15),
                    )
            for n in range(2):
                nc.vector.tensor_tensor(
                    r2_row[:, n * 512 : (n + 1) * 512],
                    ps_rn2[n],
                    rnb2[:, n * 512 : (n + 1) * 512],
                    ALU.add,
                )
            rng_row = load_row(rn_g, H, "rng_row")
            rnbeta_row = load_row(rn_beta, H, "rnbeta_row")
            ro_row = _row_layernorm(
                nc, rows, scal_p, r2_row, rng_row, rnbeta_row, eps1, "ro"
            )
            nc.sync.dma_start(out=ro_out[:, :], in_=ro_row)

            # --- validation network ---
            ro_col = _row_to_col(nc, psb_ca, one1, colp, ro_row, KO1, "roc")
            vnb1_row = load_row(vn_b1, H // 2, "vnb1_row")
            ps_v = psb_va.tile([1, 512], f32, name="ps_v", tag="vecacc")
            for ko in range(KO1):
                vw = vnwp.tile([P, 512], f32, name="vnw", tag="vnw")
                nc.sync.dma_start(out=vw, in_=vn_w1[ko * P : (ko + 1) * P, :])
                nc.tensor.matmul(
                    ps_v,
                    lhsT=ro_col[:, ko : ko + 1],
                    rhs=vw,
                    start=(ko == 0),
                    stop=(ko == KO1 - 1),
                )
            v1_row = row_tile(512, "v1_row")
            nc.vector.tensor_tensor(v1_row, ps_v, vnb1_row, ALU.add)
            nc.scalar.activation(v1_row, v1_row, AF.Gelu)
            v1_col = _row_to_col(nc, psb_ca, one1, colp, v1_row, 4, "v1")
            vnw2 = colp.tile([P, 4, 1], f32, name="vnw2", tag="vnw2")
            nc.sync.dma_start(
                out=vnw2, in_=vn_w2.rearrange("(ko p) n -> p ko n", p=P)
            )
            vnb2_row = scal_p.tile([1, 1], f32, name="vnb2_row", tag="scal")
            nc.sync.dma_start(out=vnb2_row, in_=vn_b2[:, :])
            ps_v2 = psb_va.tile([1, 1], f32, name="ps_v2", tag="vecacc")
            for ko in range(4):
                nc.tensor.matmul(
                    ps_v2,
                    lhsT=v1_col[:, ko : ko + 1],
                    rhs=vnw2[:, ko, :],
                    start=(ko == 0),
                    stop=(ko == 3),
                )
            v2_row = scal_p.tile([1, 1], f32, name="v2_row", tag="scal")
            nc.vector.tensor_tensor(v2_row, ps_v2, vnb2_row, ALU.add)
            nc.scalar.activation(v2_row, v2_row, AF.Sigmoid)
            nc.sync.dma_start(out=vs_out[:, :], in_=v2_row)

    nc.compile()
    return nc


_NC_CACHE = None


def _get_nc():
    global _NC_CACHE
    if _NC_CACHE is None:
        _NC_CACHE = build_kernel()
    return _NC_CACHE


def run(inputs, trace=False):
    from concourse.bass_utils import run_bass_kernel_spmd

    nc = _get_nc()
    hs = np.ascontiguousarray(inputs["hidden_states"], dtype=np.float32)
    B = hs.shape[0]
    assert B == N_CORES

    def row(name, n):
        return np.ascontiguousarray(
            np.asarray(inputs[name], dtype=np.float32).reshape(1, n)
        )

    E = np.ascontiguousarray(inputs["entity_embeddings"], np.float32)
    shared = {
        "ent": E,
        "entT": np.ascontiguousarray(E.T),
        "ee_w1": np.ascontiguousarray(inputs["ee_w1"], np.float32),
        "ee_b1": row("ee_b1", H2),
        "ee_w2": np.ascontiguousarray(inputs["ee_w2"], np.float32),
        "ee_b2": row("ee_b2", H),
        "ee_g": row("ee_g", H),
        "ee_beta": row("ee_beta", H),
        "re_w1": np.ascontiguousarray(inputs["re_w1"], np.float32),
        "re_b1": row("re_b1", H),
        "re_w2": np.ascontiguousarray(inputs["re_w2"], np.float32),
        "re_b2": row("re_b2", H),
        "re_g": row("re_g", H),
        "re_beta": row("re_beta", H),
        "rn_w1": np.ascontiguousarray(inputs["rn_w1"], np.float32),
        "rn_b1": row("rn_b1", H2),
        "rn_w2": np.ascontiguousarray(inputs["rn_w2"], np.float32),
        "rn_b2": row("rn_b2", H),
        "rn_g": row("rn_g", H),
        "rn_beta": row("rn_beta", H),
        "vn_w1": np.ascontiguousarray(inputs["vn_w1"], np.float32),
        "vn_b1": row("vn_b1", H // 2),
        "vn_w2": np.ascontiguousarray(inputs["vn_w2"], np.float32),
        "vn_b2": row("vn_b2", 1),
    }
    in_maps = [
        dict(shared, xT=np.ascontiguousarray(hs[c].T)) for c in range(B)
    ]
    res = run_bass_kernel_spmd(
        nc, in_maps, core_ids=list(range(N_CORES)), trace=trace
    )
    r = res.results
    ef = np.stack([r[c]["ef_out"] for c in range(B)])
    rf = np.stack([r[c]["rf_out"][0] for c in range(B)])
    ret = np.stack([r[c]["ret_out"] for c in range(B)])
    sims = np.stack([r[c]["sims_out"][0] for c in range(B)])
    idx = np.stack([r[c]["idx_out"][0] for c in range(B)]).astype(np.int32)
    ro = np.stack([r[c]["ro_out"][0] for c in range(B)])
    vs = np.stack([r[c]["vs_out"][0] for c in range(B)])
    out = (ef, rf, ret, sims, idx, ro, vs)
    return (out, res) if trace else out


def kernel(**inputs):
    return run(inputs, trace=False)
